# revision 29
# baseline (speedup 1.0000x reference)
"""DBRX MoE experts kernel for 8 Trainium2 NeuronCores.

Strategy (expert-parallel with host-side token dispatch):
  - Host computes the (cheap) router: softmax over 16 experts, top-4,
    renormalized gates.  Tokens are gathered per expert.
  - Each core gets NG=2 expert "groups" (16 experts / 8 cores).  Experts are
    sorted by token count: the 8 largest go in group 0, the 8 smallest in
    group 1, and each group's tokens are packed into MG tiles of T_g tokens
    (zero padded, T sized per group to the largest expert in it).  The
    expert's weights are loaded once per group and reused across its tiles.
  - Device (SPMD, one program on all 8 cores) runs the expert FFN:
    h = wsT.T @ x (both halves), act = silu(h1)*h2, y = w2T.T @ act.
    All matmuls in float16 (full PE speed, FWL weight loads, half the DMA
    bytes of fp32; end-to-end rel err ~1e-3 vs the 2e-2 gate).
  - Host applies gates and scatter-adds item outputs into the [T, D] output.

Perf structure:
  - Startup: the first weight pair + first x tile are split into small
    chunks and issued first across both HWDGE queues (sync/scalar) in the
    order the PE consumes them; remaining first-group x tiles go through
    gpsimd's SWDGE queue in parallel.
  - A short burst of dummy matmuls on memset data warms the PE clock (HAM)
    while the startup DMAs are still in flight.
  - Both groups' x/act tiles are held in SBUF simultaneously (no slot
    reuse), so group 1's loads overlap group 0's compute and the PE never
    idles at the group boundary.
  - Output tiles are stored with 2 chunked DMAs alternating queues to
    shorten the drain tail.

Self-contained: hardcodes T=4096 tokens, D=1024, I=2048, E=16, top_k=4,
8 cores.
"""

import sys

if "/opt/trn_rl_repo" not in sys.path:
    sys.path.insert(0, "/opt/trn_rl_repo")

import numpy as np

import concourse.bacc as bacc
import concourse.mybir as mybir
import concourse.tile as tile
from concourse.bass_utils import run_bass_kernel_spmd

TOP_K = 4
N_CORES = 8
D = 1024
I = 2048
E = 16
DC = D // 128  # 8 contraction chunks for mm1 / output blocks for mm2
IC = I // 128  # 16 intermediate blocks
CB = 2 * I // 128  # 32 column blocks of ws

TRACE = False
LAST_EXEC_NS = None
WARMUP_MMS = 17  # dummy N=256 matmuls at t=0 to flip the HAM clock gate early

_compiled = {}  # shapes tuple -> nc


def _build_program(shapes):
    """shapes: tuple of (MG, T) per group (one group = one expert)."""
    f16 = mybir.dt.float16
    f32 = mybir.dt.float32
    NG = len(shapes)
    NT = sum(mg for mg, _ in shapes)
    nc = bacc.Bacc("TRN2", target_bir_lowering=False, debug=False, num_devices=N_CORES)

    xTs, yTs = [], []
    for g, (MG, T) in enumerate(shapes):
        xTs.append(
            nc.dram_tensor(f"xT{g}", [MG, 128, DC, T], f16, kind="ExternalInput")
        )
        yTs.append(
            nc.dram_tensor(f"yT{g}", [MG, DC, 128, T], f16, kind="ExternalOutput")
        )
    wsT = nc.dram_tensor("wsT", [NG, CB, 128, DC, 128], f16, kind="ExternalInput")
    w2T = nc.dram_tensor("w2T", [NG, DC, 128, IC, 128], f16, kind="ExternalInput")

    with tile.TileContext(nc) as tc:
        with (
            tc.tile_pool(name="xp", bufs=NT) as xp,
            tc.tile_pool(name="wp", bufs=6) as wp,
            tc.tile_pool(name="w2p", bufs=3) as w2p,
            tc.tile_pool(name="actp", bufs=NT) as actp,
            tc.tile_pool(name="sp", bufs=3) as sp,
            tc.tile_pool(name="warmp", bufs=1) as warmp,
            tc.tile_pool(name="pp", bufs=6, space="PSUM") as pp,
            tc.tile_pool(name="pp2", bufs=2, space="PSUM") as pp2,
        ):
            Tmax = max(t for _, t in shapes)

            # --- PE warmup: dummy matmuls on memset data flip the HAM
            # clock gate (1.2 -> 2.4 GHz needs ~3.4us of sustained PE
            # activity) while the startup DMAs are still in flight.  The
            # burst is sized to end right as the first real tiles land so
            # the PE busy-streak is unbroken and real MMs start warm.
            if WARMUP_MMS:
                wx = warmp.tile([128, 256], f16, tag="wx", name="warm_x")
                nc.vector.memset(wx[:], 0)
                wps = pp2.tile([128, 512], f32, tag="y", name="warm_ps")
                for _ in range(WARMUP_MMS):
                    nc.tensor.matmul(
                        wps[:, :256], wx[:, :128], wx[:], start=True, stop=True
                    )

            def soak(n):
                # dummy matmuls emitted at known startup DMA cliffs: they
                # keep the PE busy-streak alive (HAM stays at full clock)
                # while a late tile lands, at ~109ns each when not needed
                if WARMUP_MMS:
                    for _ in range(n):
                        nc.tensor.matmul(
                            wps[:, :256], wx[:, :128], wx[:], start=True, stop=True
                        )

            # --- startup-critical loads, in PE consumption order, spread
            # over both HWDGE queues.  MM order is ps1(dc=0..7) with w1,
            # then ps2(dc=0..7) with v1, j tiles outer; so: w1/x0 chunks
            # first, then v1, then x1/x2, then the ip>=1 weight stream.
            MG0, T0 = shapes[0]
            w1t0 = wp.tile([128, DC, 128], f16, tag="ws0", bufs=2, name="w1t0_g0")
            v1t0 = wp.tile([128, DC, 128], f16, tag="ws0", bufs=2, name="v1t0_g0")
            xtiles = [[None] * mg for mg, _ in shapes]
            for j in range(MG0):
                xtiles[0][j] = xp.tile(
                    [128, DC, Tmax], f16, tag="x", bufs=NT, name=f"x_0_{j}"
                )[:, :, :T0]
            # 8 startup DMAs total, ordered by PE need time per queue; the
            # HWDGE ring paces issues (~1.3us apart after the first few),
            # so fewer/larger transfers land the critical tiles sooner.
            # the last j tile of group 0 is deferred to a second mm1 pass
            # (weights re-streamed; DMA has slack there), so the startup
            # window only has to land the w pair + x0 + x1
            xt00 = xtiles[0][0]
            nc.sync.dma_start(w1t0[:, :4], wsT.ap()[0, 0][:, :4])
            nc.scalar.dma_start(xt00[:, 0:4], xTs[0].ap()[0, :, 0:4])
            nc.sync.dma_start(xt00[:, 4:8], xTs[0].ap()[0, :, 4:8])
            nc.scalar.dma_start(w1t0[:, 4:], wsT.ap()[0, 0][:, 4:])
            nc.sync.dma_start(v1t0[:], wsT.ap()[0, IC])
            if MG0 > 1:
                nc.scalar.dma_start(xtiles[0][1][:], xTs[0].ap()[1])

            acts = [[None] * mg for mg, _ in shapes]
            for g, (MG, T) in enumerate(shapes):
                for j in range(MG):
                    acts[g][j] = actp.tile(
                        [128, IC, Tmax], f16, tag="act", bufs=NT, name=f"act_{g}_{j}"
                    )[:, :, :T]

            qs = [nc.sync, nc.scalar]

            def mm1(g, T, j_list, w1_first=None, v1_first=None):
                for ip in range(IC):
                    if ip == 0 and w1_first is not None:
                        w1t, v1t = w1_first, v1_first
                    else:
                        w1t = wp.tile([128, DC, 128], f16, tag="ws", bufs=4)
                        qs[ip % 2].dma_start(w1t[:], wsT.ap()[g, ip])
                        v1t = wp.tile([128, DC, 128], f16, tag="ws", bufs=4)
                        qs[(ip + 1) % 2].dma_start(v1t[:], wsT.ap()[g, IC + ip])
                    for j in j_list:
                        ps1 = pp.tile([128, T], f32, tag="h")
                        ps2 = pp.tile([128, T], f32, tag="h")
                        for dc in range(DC):
                            nc.tensor.matmul(
                                ps1[:], w1t[:, dc], xtiles[g][j][:, dc],
                                start=(dc == 0), stop=(dc == DC - 1),
                            )
                        for dc in range(DC):
                            nc.tensor.matmul(
                                ps2[:], v1t[:, dc], xtiles[g][j][:, dc],
                                start=(dc == 0), stop=(dc == DC - 1),
                            )
                        st = sp.tile([128, T], f32, tag="silu")
                        nc.scalar.activation(
                            st[:], ps1[:], mybir.ActivationFunctionType.Silu
                        )
                        nc.vector.tensor_mul(acts[g][j][:, ip], st[:], ps2[:])

            def mm2(g, MG, T, last_group=False):
                h = T // 2
                for db in range(DC):
                    w2t = w2p.tile([128, IC, 128], f16, tag="w2")
                    qs[db % 2].dma_start(w2t[:, :8], w2T.ap()[g, db][:, :8])
                    qs[(db + 1) % 2].dma_start(w2t[:, 8:], w2T.ap()[g, db][:, 8:])
                    for j in range(MG):
                        if last_group and db == DC - 1 and j == MG - 1:
                            # final output tile: accumulate the two column
                            # halves in separate banks so the first half
                            # drains while the second computes -> short tail
                            for k in range(2):
                                psk = pp2.tile([128, Tmax], f32, tag="y")
                                for ic in range(IC):
                                    nc.tensor.matmul(
                                        psk[:, :h],
                                        w2t[:, ic],
                                        acts[g][j][:, ic, k * h : (k + 1) * h],
                                        start=(ic == 0), stop=(ic == IC - 1),
                                    )
                                ot = sp.tile([128, Tmax // 2], f16, tag="yh2", bufs=2)
                                nc.any.tensor_copy(ot[:, :h], psk[:, :h])
                                qs[k].dma_start(
                                    yTs[g].ap()[j, db][:, k * h : (k + 1) * h],
                                    ot[:, :h],
                                )
                            continue
                        ps3 = pp2.tile([128, T], f32, tag="y")
                        for ic in range(IC):
                            nc.tensor.matmul(
                                ps3[:], w2t[:, ic], acts[g][j][:, ic],
                                start=(ic == 0), stop=(ic == IC - 1),
                            )
                        # one fp32 PSUM -> fp16 SBUF copy, then two
                        # half-stores that fire in parallel on both queues
                        ot = sp.tile([128, Tmax], f16, tag="yout", bufs=3)
                        nc.any.tensor_copy(ot[:, :T], ps3[:])
                        qs[j % 2].dma_start(yTs[g].ap()[j, db][:, :h], ot[:, :h])
                        qs[(j + 1) % 2].dma_start(yTs[g].ap()[j, db][:, h:], ot[:, h:T])

            # group 0 FFN.  When the group has 3+ tiles, the last one runs
            # as a second pass (weights re-streamed) so its x load stays
            # out of the congested startup window.
            if MG0 > 2:
                mm1(0, T0, list(range(MG0 - 1)), w1t0, v1t0)
                nc.sync.dma_start(xtiles[0][MG0 - 1][:], xTs[0].ap()[MG0 - 1])
                mm1(0, T0, [MG0 - 1])
            else:
                mm1(0, T0, list(range(MG0)), w1t0, v1t0)

            # prefetch group 1 inputs during group-0 mm2: x tiles (HWDGE
            # queues, priority after group-0 mm1 issues) + first weight pair
            # (dedicated "ws0" slots, free since early mm1).
            if NG > 1:
                MG1, T1 = shapes[1]
                for j in range(MG1):
                    xt = xp.tile(
                        [128, DC, Tmax], f16, tag="x", bufs=NT, name=f"x_1_{j}"
                    )[:, :, :T1]
                    xtiles[1][j] = xt
                    qs[j % 2].dma_start(xt[:, 0:4], xTs[1].ap()[j, :, 0:4])
                    qs[(j + 1) % 2].dma_start(xt[:, 4:8], xTs[1].ap()[j, :, 4:8])
                w1t0_g1 = wp.tile([128, DC, 128], f16, tag="ws0", bufs=2, name="w1t0_g1")
                nc.sync.dma_start(w1t0_g1[:], wsT.ap()[1, 0])
                v1t0_g1 = wp.tile([128, DC, 128], f16, tag="ws0", bufs=2, name="v1t0_g1")
                nc.scalar.dma_start(v1t0_g1[:], wsT.ap()[1, IC])

            mm2(0, MG0, T0)

            if NG > 1:
                mm1(1, T1, list(range(MG1)), w1t0_g1, v1t0_g1)
                mm2(1, MG1, T1)
    nc.compile()
    return nc


def _routing(x, rw):
    logits = x @ rw.T
    m = logits.max(-1, keepdims=True)
    p = np.exp(logits - m)
    p /= p.sum(-1, keepdims=True)
    topk_idx = np.argpartition(-p, TOP_K - 1, axis=-1)[:, :TOP_K]
    topk_val = np.take_along_axis(p, topk_idx, -1)
    topk_val = topk_val / topk_val.sum(-1, keepdims=True)
    return topk_idx, topk_val


def _group_shape(cmax):
    """Pick (MG, T) so MG*T >= cmax, T in [256, 512], minimizing MG*T."""
    best = None
    for MG in range(1, 17):
        T = -(-cmax // MG) if cmax else 256
        T = (T + 7) // 8 * 8
        if T > 512:
            continue
        T = max(T, 256)
        if best is None or MG * T < best[0]:
            best = (MG * T, MG, T)
    assert best is not None
    return best[1], best[2]


def _tile_ws(ws_e):
    # [cb, p, dc, col] = ws_e[cb*128+col, dc*128+p]
    return np.ascontiguousarray(
        ws_e.reshape(CB, 128, DC, 128).transpose(0, 3, 2, 1)
    )


def _tile_w2(w2_e):
    # [db, p, ic, col] = w2_e[db*128+col, ic*128+p]
    return np.ascontiguousarray(
        w2_e.reshape(DC, 128, IC, 128).transpose(0, 3, 2, 1)
    )


def kernel(hidden_states, router_w, ws, w2s):
    global LAST_EXEC_NS
    x = np.ascontiguousarray(np.asarray(hidden_states, dtype=np.float32))
    rw = np.asarray(router_w, dtype=np.float32)
    ws = np.asarray(ws, dtype=np.float32)
    w2s = np.asarray(w2s, dtype=np.float32)
    T_tok = x.shape[0]

    topk_idx, topk_val = _routing(x, rw)

    expert_tok = []
    expert_gate = []
    for e in range(E):
        hit = topk_idx == e
        rows = np.nonzero(hit.any(-1))[0]
        gv = np.where(hit[rows], topk_val[rows], 0.0).sum(-1).astype(np.float32)
        expert_tok.append(rows)
        expert_gate.append(gv)

    counts = np.array([len(t) for t in expert_tok])
    NG = -(-E // N_CORES)  # 2
    order = np.argsort(-counts, kind="stable")
    groups = [order[g * N_CORES : (g + 1) * N_CORES] for g in range(NG)]
    shapes = tuple(
        _group_shape(int(counts[grp].max()) if len(grp) else 0) for grp in groups
    )

    if shapes not in _compiled:
        _compiled[shapes] = _build_program(shapes)
    nc = _compiled[shapes]

    x16 = x.astype(np.float16)
    in_maps = []
    for c in range(N_CORES):
        m = {}
        wsT_b = np.empty((NG, CB, 128, DC, 128), dtype=np.float16)
        w2T_b = np.empty((NG, DC, 128, IC, 128), dtype=np.float16)
        for g, (MG, T) in enumerate(shapes):
            e = int(groups[g][c])
            wsT_b[g] = _tile_ws(ws[e].astype(np.float16))
            w2T_b[g] = _tile_w2(w2s[e].astype(np.float16))
            xT_b = np.zeros((MG, 128, DC, T), dtype=np.float16)
            toks = expert_tok[e]
            for j in range(MG):
                seg = toks[j * T : (j + 1) * T]
                n = len(seg)
                if n == 0:
                    continue
                xT_b[j, :, :, :n] = x16[seg].reshape(n, DC, 128).transpose(2, 1, 0)
            m[f"xT{g}"] = xT_b
        m["wsT"] = wsT_b
        m["w2T"] = w2T_b
        in_maps.append(m)

    res = run_bass_kernel_spmd(
        nc, in_maps, core_ids=list(range(N_CORES)), trace=TRACE
    )
    LAST_EXEC_NS = res.exec_time_ns

    out = np.zeros((T_tok, D), dtype=np.float32)
    for g, (MG, T) in enumerate(shapes):
        for c in range(N_CORES):
            e = int(groups[g][c])
            toks = expert_tok[e]
            gates = expert_gate[e]
            yT_c = res.results[c][f"yT{g}"]
            for j in range(MG):
                seg = toks[j * T : (j + 1) * T]
                n = len(seg)
                if n == 0:
                    break
                y_item = yT_c[j].transpose(2, 0, 1).reshape(T, D)[:n]
                out[seg] += gates[j * T : (j + 1) * T][:, None] * y_item
    return out


# revision 31
# speedup vs baseline: 1.0391x; 1.0391x over previous
"""DBRX MoE experts kernel for 8 Trainium2 NeuronCores.

Strategy (expert-parallel with host-side token dispatch):
  - Host computes the (cheap) router: softmax over 16 experts, top-4,
    renormalized gates.  Tokens are gathered per expert.
  - Each core gets NG=2 expert "groups" (16 experts / 8 cores).  Experts are
    sorted by token count: the 8 largest go in group 0, the 8 smallest in
    group 1, and each group's tokens are packed into MG tiles of T_g tokens
    (zero padded, T sized per group to the largest expert in it).  The
    expert's weights are loaded once per group and reused across its tiles.
  - Device (SPMD, one program on all 8 cores) runs the expert FFN:
    h = wsT.T @ x (both halves), act = silu(h1)*h2, y = w2T.T @ act.
    All matmuls in float16 (full PE speed, FWL weight loads, half the DMA
    bytes of fp32; end-to-end rel err ~1e-3 vs the 2e-2 gate).
  - Host applies gates and scatter-adds item outputs into the [T, D] output.

Perf structure:
  - Startup: the first weight pair + first x tile are split into small
    chunks and issued first across both HWDGE queues (sync/scalar) in the
    order the PE consumes them; remaining first-group x tiles go through
    gpsimd's SWDGE queue in parallel.
  - A short burst of dummy matmuls on memset data warms the PE clock (HAM)
    while the startup DMAs are still in flight.
  - Both groups' x/act tiles are held in SBUF simultaneously (no slot
    reuse), so group 1's loads overlap group 0's compute and the PE never
    idles at the group boundary.
  - Output tiles are stored with 2 chunked DMAs alternating queues to
    shorten the drain tail.

Self-contained: hardcodes T=4096 tokens, D=1024, I=2048, E=16, top_k=4,
8 cores.
"""

import sys

if "/opt/trn_rl_repo" not in sys.path:
    sys.path.insert(0, "/opt/trn_rl_repo")

import numpy as np

import concourse.bacc as bacc
import concourse.mybir as mybir
import concourse.tile as tile
from concourse.bass_utils import run_bass_kernel_spmd

TOP_K = 4
N_CORES = 8
D = 1024
I = 2048
E = 16
DC = D // 128  # 8 contraction chunks for mm1 / output blocks for mm2
IC = I // 128  # 16 intermediate blocks
CB = 2 * I // 128  # 32 column blocks of ws

TRACE = False
LAST_EXEC_NS = None
WARMUP_MMS = 17  # dummy N=256 matmuls at t=0 to flip the HAM clock gate early

_compiled = {}  # shapes tuple -> nc


def _build_program(shapes):
    """shapes: tuple of (MG, T) per group (one group = one expert)."""
    f16 = mybir.dt.float16
    f32 = mybir.dt.float32
    NG = len(shapes)
    NT = sum(mg for mg, _ in shapes)
    nc = bacc.Bacc("TRN2", target_bir_lowering=False, debug=False, num_devices=N_CORES)

    xTs, yTs = [], []
    for g, (MG, T) in enumerate(shapes):
        xTs.append(
            nc.dram_tensor(f"xT{g}", [MG, 128, DC, T], f16, kind="ExternalInput")
        )
        yTs.append(
            nc.dram_tensor(f"yT{g}", [MG, DC, 128, T], f16, kind="ExternalOutput")
        )
    wsT = nc.dram_tensor("wsT", [NG, CB, 128, DC, 128], f16, kind="ExternalInput")
    w2T = nc.dram_tensor("w2T", [NG, DC, 128, IC, 128], f16, kind="ExternalInput")

    with tile.TileContext(nc) as tc:
        with (
            tc.tile_pool(name="xp", bufs=NT) as xp,
            tc.tile_pool(name="wp", bufs=6) as wp,
            tc.tile_pool(name="w2p", bufs=3) as w2p,
            tc.tile_pool(name="actp", bufs=NT) as actp,
            tc.tile_pool(name="sp", bufs=3) as sp,
            tc.tile_pool(name="warmp", bufs=1) as warmp,
            tc.tile_pool(name="pp", bufs=6, space="PSUM") as pp,
            tc.tile_pool(name="pp2", bufs=2, space="PSUM") as pp2,
        ):
            Tmax = max(t for _, t in shapes)

            # --- PE warmup: dummy matmuls on memset data flip the HAM
            # clock gate (1.2 -> 2.4 GHz needs ~3.4us of sustained PE
            # activity) while the startup DMAs are still in flight.  The
            # burst is sized to end right as the first real tiles land so
            # the PE busy-streak is unbroken and real MMs start warm.
            if WARMUP_MMS:
                wx = warmp.tile([128, 256], f16, tag="wx", name="warm_x")
                nc.vector.memset(wx[:], 0)
                wps = pp2.tile([128, 512], f32, tag="y", name="warm_ps")
                for _ in range(WARMUP_MMS):
                    nc.tensor.matmul(
                        wps[:, :256], wx[:, :128], wx[:], start=True, stop=True
                    )

            def soak(n):
                # dummy matmuls emitted at known startup DMA cliffs: they
                # keep the PE busy-streak alive (HAM stays at full clock)
                # while a late tile lands, at ~109ns each when not needed
                if WARMUP_MMS:
                    for _ in range(n):
                        nc.tensor.matmul(
                            wps[:, :256], wx[:, :128], wx[:], start=True, stop=True
                        )

            # --- startup-critical loads, in PE consumption order, spread
            # over both HWDGE queues.  MM order is ps1(dc=0..7) with w1,
            # then ps2(dc=0..7) with v1, j tiles outer; so: w1/x0 chunks
            # first, then v1, then x1/x2, then the ip>=1 weight stream.
            MG0, T0 = shapes[0]
            w1t0 = wp.tile([128, DC, 128], f16, tag="ws0", bufs=2, name="w1t0_g0")
            v1t0 = wp.tile([128, DC, 128], f16, tag="ws0", bufs=2, name="v1t0_g0")
            xtiles = [[None] * mg for mg, _ in shapes]
            for j in range(MG0):
                xtiles[0][j] = xp.tile(
                    [128, DC, Tmax], f16, tag="x", bufs=NT, name=f"x_0_{j}"
                )[:, :, :T0]
            # 8 startup DMAs total, ordered by PE need time per queue; the
            # HWDGE ring paces issues (~1.3us apart after the first few),
            # so fewer/larger transfers land the critical tiles sooner.
            xt00 = xtiles[0][0]
            nc.sync.dma_start(w1t0[:, :4], wsT.ap()[0, 0][:, :4])
            nc.scalar.dma_start(xt00[:, 0:4], xTs[0].ap()[0, :, 0:4])
            nc.sync.dma_start(w1t0[:, 4:], wsT.ap()[0, 0][:, 4:])
            nc.scalar.dma_start(v1t0[:], wsT.ap()[0, IC])
            nc.sync.dma_start(xt00[:, 4:8], xTs[0].ap()[0, :, 4:8])
            if MG0 > 1:
                nc.scalar.dma_start(xtiles[0][1][:], xTs[0].ap()[1])
            for j in range(2, MG0):
                nc.sync.dma_start(xtiles[0][j][:], xTs[0].ap()[j])

            acts = [[None] * mg for mg, _ in shapes]
            for g, (MG, T) in enumerate(shapes):
                for j in range(MG):
                    acts[g][j] = actp.tile(
                        [128, IC, Tmax], f16, tag="act", bufs=NT, name=f"act_{g}_{j}"
                    )[:, :, :T]

            qs = [nc.sync, nc.scalar]

            def mm1(g, T, j_list, w1_first=None, v1_first=None):
                for ip in range(IC):
                    if ip == 0 and w1_first is not None:
                        w1t, v1t = w1_first, v1_first
                    else:
                        w1t = wp.tile([128, DC, 128], f16, tag="ws", bufs=4)
                        qs[ip % 2].dma_start(w1t[:], wsT.ap()[g, ip])
                        v1t = wp.tile([128, DC, 128], f16, tag="ws", bufs=4)
                        qs[(ip + 1) % 2].dma_start(v1t[:], wsT.ap()[g, IC + ip])
                    for j in j_list:
                        ps1 = pp.tile([128, T], f32, tag="h")
                        ps2 = pp.tile([128, T], f32, tag="h")
                        for dc in range(DC):
                            nc.tensor.matmul(
                                ps1[:], w1t[:, dc], xtiles[g][j][:, dc],
                                start=(dc == 0), stop=(dc == DC - 1),
                            )
                        for dc in range(DC):
                            nc.tensor.matmul(
                                ps2[:], v1t[:, dc], xtiles[g][j][:, dc],
                                start=(dc == 0), stop=(dc == DC - 1),
                            )
                        st = sp.tile([128, T], f32, tag="silu")
                        nc.scalar.activation(
                            st[:], ps1[:], mybir.ActivationFunctionType.Silu
                        )
                        nc.vector.tensor_mul(acts[g][j][:, ip], st[:], ps2[:])

            def mm2(g, MG, T, last_group=False):
                h = T // 2
                for db in range(DC):
                    w2t = w2p.tile([128, IC, 128], f16, tag="w2")
                    qs[db % 2].dma_start(w2t[:, :8], w2T.ap()[g, db][:, :8])
                    qs[(db + 1) % 2].dma_start(w2t[:, 8:], w2T.ap()[g, db][:, 8:])
                    for j in range(MG):
                        if last_group and db == DC - 1 and j == MG - 1:
                            # final output tile: accumulate the two column
                            # halves in separate banks so the first half
                            # drains while the second computes -> short tail
                            for k in range(2):
                                psk = pp2.tile([128, Tmax], f32, tag="y")
                                for ic in range(IC):
                                    nc.tensor.matmul(
                                        psk[:, :h],
                                        w2t[:, ic],
                                        acts[g][j][:, ic, k * h : (k + 1) * h],
                                        start=(ic == 0), stop=(ic == IC - 1),
                                    )
                                ot = sp.tile([128, Tmax // 2], f16, tag="yh2", bufs=2)
                                nc.any.tensor_copy(ot[:, :h], psk[:, :h])
                                qs[k].dma_start(
                                    yTs[g].ap()[j, db][:, k * h : (k + 1) * h],
                                    ot[:, :h],
                                )
                            continue
                        ps3 = pp2.tile([128, T], f32, tag="y")
                        for ic in range(IC):
                            nc.tensor.matmul(
                                ps3[:], w2t[:, ic], acts[g][j][:, ic],
                                start=(ic == 0), stop=(ic == IC - 1),
                            )
                        # one fp32 PSUM -> fp16 SBUF copy, then two
                        # half-stores that fire in parallel on both queues
                        ot = sp.tile([128, Tmax], f16, tag="yout", bufs=3)
                        nc.any.tensor_copy(ot[:, :T], ps3[:])
                        qs[j % 2].dma_start(yTs[g].ap()[j, db][:, :h], ot[:, :h])
                        qs[(j + 1) % 2].dma_start(yTs[g].ap()[j, db][:, h:], ot[:, h:T])

            # group 0 FFN
            mm1(0, T0, list(range(MG0)), w1t0, v1t0)

            # prefetch group 1 inputs during group-0 mm2: x tiles (HWDGE
            # queues, priority after group-0 mm1 issues) + first weight pair
            # (dedicated "ws0" slots, free since early mm1).
            if NG > 1:
                MG1, T1 = shapes[1]
                for j in range(MG1):
                    xt = xp.tile(
                        [128, DC, Tmax], f16, tag="x", bufs=NT, name=f"x_1_{j}"
                    )[:, :, :T1]
                    xtiles[1][j] = xt
                    qs[j % 2].dma_start(xt[:, 0:4], xTs[1].ap()[j, :, 0:4])
                    qs[(j + 1) % 2].dma_start(xt[:, 4:8], xTs[1].ap()[j, :, 4:8])
                w1t0_g1 = wp.tile([128, DC, 128], f16, tag="ws0", bufs=2, name="w1t0_g1")
                nc.sync.dma_start(w1t0_g1[:], wsT.ap()[1, 0])
                v1t0_g1 = wp.tile([128, DC, 128], f16, tag="ws0", bufs=2, name="v1t0_g1")
                nc.scalar.dma_start(v1t0_g1[:], wsT.ap()[1, IC])

            mm2(0, MG0, T0)

            if NG > 1:
                mm1(1, T1, list(range(MG1)), w1t0_g1, v1t0_g1)
                mm2(1, MG1, T1)
    nc.compile()
    return nc


def _routing(x, rw):
    logits = x @ rw.T
    m = logits.max(-1, keepdims=True)
    p = np.exp(logits - m)
    p /= p.sum(-1, keepdims=True)
    topk_idx = np.argpartition(-p, TOP_K - 1, axis=-1)[:, :TOP_K]
    topk_val = np.take_along_axis(p, topk_idx, -1)
    topk_val = topk_val / topk_val.sum(-1, keepdims=True)
    return topk_idx, topk_val


def _group_shape(cmax):
    """Pick (MG, T) so MG*T >= cmax, T in [256, 512], minimizing MG*T."""
    best = None
    for MG in range(1, 17):
        T = -(-cmax // MG) if cmax else 256
        T = (T + 7) // 8 * 8
        if T > 512:
            continue
        T = max(T, 256)
        if best is None or MG * T < best[0]:
            best = (MG * T, MG, T)
    assert best is not None
    return best[1], best[2]


def _tile_ws(ws_e):
    # [cb, p, dc, col] = ws_e[cb*128+col, dc*128+p]
    return np.ascontiguousarray(
        ws_e.reshape(CB, 128, DC, 128).transpose(0, 3, 2, 1)
    )


def _tile_w2(w2_e):
    # [db, p, ic, col] = w2_e[db*128+col, ic*128+p]
    return np.ascontiguousarray(
        w2_e.reshape(DC, 128, IC, 128).transpose(0, 3, 2, 1)
    )


def kernel(hidden_states, router_w, ws, w2s):
    global LAST_EXEC_NS
    x = np.ascontiguousarray(np.asarray(hidden_states, dtype=np.float32))
    rw = np.asarray(router_w, dtype=np.float32)
    ws = np.asarray(ws, dtype=np.float32)
    w2s = np.asarray(w2s, dtype=np.float32)
    T_tok = x.shape[0]

    topk_idx, topk_val = _routing(x, rw)

    expert_tok = []
    expert_gate = []
    for e in range(E):
        hit = topk_idx == e
        rows = np.nonzero(hit.any(-1))[0]
        gv = np.where(hit[rows], topk_val[rows], 0.0).sum(-1).astype(np.float32)
        expert_tok.append(rows)
        expert_gate.append(gv)

    counts = np.array([len(t) for t in expert_tok])
    NG = -(-E // N_CORES)  # 2
    order = np.argsort(-counts, kind="stable")
    groups = [order[g * N_CORES : (g + 1) * N_CORES] for g in range(NG)]
    shapes = tuple(
        _group_shape(int(counts[grp].max()) if len(grp) else 0) for grp in groups
    )

    if shapes not in _compiled:
        _compiled[shapes] = _build_program(shapes)
    nc = _compiled[shapes]

    x16 = x.astype(np.float16)
    in_maps = []
    for c in range(N_CORES):
        m = {}
        wsT_b = np.empty((NG, CB, 128, DC, 128), dtype=np.float16)
        w2T_b = np.empty((NG, DC, 128, IC, 128), dtype=np.float16)
        for g, (MG, T) in enumerate(shapes):
            e = int(groups[g][c])
            wsT_b[g] = _tile_ws(ws[e].astype(np.float16))
            w2T_b[g] = _tile_w2(w2s[e].astype(np.float16))
            xT_b = np.zeros((MG, 128, DC, T), dtype=np.float16)
            toks = expert_tok[e]
            for j in range(MG):
                seg = toks[j * T : (j + 1) * T]
                n = len(seg)
                if n == 0:
                    continue
                xT_b[j, :, :, :n] = x16[seg].reshape(n, DC, 128).transpose(2, 1, 0)
            m[f"xT{g}"] = xT_b
        m["wsT"] = wsT_b
        m["w2T"] = w2T_b
        in_maps.append(m)

    res = run_bass_kernel_spmd(
        nc, in_maps, core_ids=list(range(N_CORES)), trace=TRACE
    )
    LAST_EXEC_NS = res.exec_time_ns

    out = np.zeros((T_tok, D), dtype=np.float32)
    for g, (MG, T) in enumerate(shapes):
        for c in range(N_CORES):
            e = int(groups[g][c])
            toks = expert_tok[e]
            gates = expert_gate[e]
            yT_c = res.results[c][f"yT{g}"]
            for j in range(MG):
                seg = toks[j * T : (j + 1) * T]
                n = len(seg)
                if n == 0:
                    break
                y_item = yT_c[j].transpose(2, 0, 1).reshape(T, D)[:n]
                out[seg] += gates[j * T : (j + 1) * T][:, None] * y_item
    return out
